# revision 61
# baseline (speedup 1.0000x reference)
"""BitLinear (ternary weight quantization + linear) on 8 TRN2 NeuronCores.

y = x @ w_eff.T with w_eff = clip(round(w/scale), -1, 1) * scale,
scale = clamp(mean |w| per row, 1e-5).

Sharding: column-parallel — weight rows (out_features) split 8 ways; each
core computes y[:, shard] for the full x; host concatenates. Quantization
is per-output-row, so it is fully local to a shard.

Design (evolved from a transpose-on-device fp32r baseline at 634us):
  * The host pre-permutes x into bf16 [16 chunks][128 k_in][4 msub]
    [16 k_sub][128 rows] so every x tile lands in SBUF ready to use as
    the matmul stationary operand — the device does ZERO x preprocessing
    (the baseline spent ~86us of PE time transposing x via PSUM).
  * bf16 matmuls: w_eff is ternary*scale (bf16-exact up to a coherent
    0.4% per-row scale rounding), x is bf16 (~0.2% element error). bf16
    enables fast weight load, so the per-matmul LDWEIGHTS (~97ns) hides
    completely under the 216ns matmul issue gap — the steady-state
    stream runs at the PE floor (512 cycles @2.4GHz per 128x512 MM).
  * W-phase quantization math is bit-identical to the jax reference
    (Abs row-sum on ACT, clamped scale, is_gt/is_lt ternary build on
    DVE): round(w/scale) at the 0.5 boundary must match exactly — one
    flipped ternary weight costs 1.35e-2 absmax error (2/3 of budget).
    All abs ops for a W group are emitted before any W eviction so the
    ACT FIFO never blocks an abs behind an eviction that waits on PE
    transposes (which wait on DVE quant — an 8.3us/chunk spiral).
  * DMA ring assignment matters: x sub-DMAs go on the gpsimd/SWDGE path
    (HWDGE rings execute DMAs in FIFO order per ring, so sharing the
    sync ring with y stores head-of-line-blocks them; issuing on the
    scalar ring stalls ACT evictions). w/y use the sync+scalar HWDGE
    rings. PSUM evictions split: n=0 groups on ACT, n=1 on DVE.
  * Warm-up matmuls bridge the PE through the W-phase lead-in so the
    HAM clock gate (1.2GHz cold / 2.4GHz warm) is at 8/8 when the real
    stream starts.

Per-core steady state: 16 row-chunks of 512; per chunk four 512KiB x
sub-DMAs, then 8 accumulation groups (4 m-subtiles x 2 n-slices) of 16
matmuls [128x128]@[128x512] into one PSUM bank each, eviction, 256KB y
DMA per group.
"""

import ml_dtypes
import numpy as np

import concourse.bass as bass
import concourse.mybir as mybir
import concourse.tile as tile
from concourse import bacc
from concourse.bass_utils import run_bass_kernel_spmd
from concourse.masks import make_identity

F32 = mybir.dt.float32
F32R = mybir.dt.float32r
BF16 = mybir.dt.bfloat16

# Problem shape (hardcoded per contract)
B, S, D_IN, D_OUT = 4, 2048, 2048, 8192
NCORES = 8
R = B * S                 # 8192 rows of x
O = D_OUT // NCORES       # 1024 out features per core
K_SUB = D_IN // 128       # 16 contraction sub-tiles
O_TILES = O // 128        # 8 weight row-tiles per core
N_SLICE = 512             # psum bank width (fp32)
N_SLICES = O // N_SLICE   # 2
TGRP = 4                  # transposes batched per psum bank
RCHUNK = 512              # x rows per streamed chunk
NCHUNK = R // RCHUNK      # 16
MSUB = RCHUNK // 128      # 4


def _build():
    nc = bacc.Bacc(None, target_bir_lowering=False)

    x_d = nc.dram_tensor("x", [NCHUNK, 128, MSUB, K_SUB, 128], BF16,
                         kind="ExternalInput")
    w_d = nc.dram_tensor("w", [O, D_IN], F32, kind="ExternalInput")
    y_d = nc.dram_tensor("y", [R, O], F32, kind="ExternalOutput")

    with tile.TileContext(nc) as tc:
        with (
            tc.tile_pool(name="const", bufs=1) as const,
            tc.tile_pool(name="wt", bufs=1) as wtp,
            tc.tile_pool(name="ws", bufs=1) as ws,
            tc.tile_pool(name="xs", bufs=1) as xs,
            tc.tile_pool(name="ys", bufs=1) as ysp,
            tc.tile_pool(name="ps", bufs=2, space="PSUM") as ps,
            tc.tile_pool(name="ymm", bufs=1, space="PSUM") as ymm,
        ):
            # HAM warm-up fodder; the accumulator borrows a y_ps slot so
            # all 8 PSUM banks serve the matmul stream in steady state
            # (6 acc + 2 transpose staging)
            dummy = const.tile([128, N_SLICE], BF16)
            nc.vector.memset(dummy[:], 0.0)
            wacc = ymm.tile([128, N_SLICE], F32, tag="warm", bufs=1)

            def warmup(n):
                for _ in range(n):
                    nc.tensor.matmul(wacc[:], dummy[:, :128], dummy[:],
                                     start=True, stop=True)

            ident_f = const.tile([128, 128], F32)
            make_identity(nc, ident_f[:])
            ident = const.tile([128, 128], BF16)
            nc.vector.tensor_copy(ident[:], ident_f[:])

            # W^T resident in SBUF (bf16), one tile per n-slice:
            # wts[n][:, k, o'] = w_eff^T[k_in, k_sub, n*512 + o']
            wts = [
                wtp.tile([128, K_SUB, N_SLICE], BF16, name=f"wt{n}")
                for n in range(N_SLICES)
            ]

            w_tiles = {}
            w_scales = {}

            def w_dma(a):
                """Start the DMA for weight rows a*128..(a+1)*128,
                alternating between the two HWDGE rings so transfers
                overlap pairwise."""
                w_in = ws.tile([128, D_IN], F32, tag="w_in", bufs=4,
                               name=f"w_in_{a}")
                eng = nc.sync if a % 2 == 0 else nc.scalar
                eng.dma_start(w_in[:], w_d[a * 128 : (a + 1) * 128, :])
                w_tiles[a] = w_in

            def w_abs(a):
                """|w| row-sum -> clamped scale for W chunk a (ACT + tiny
                DVE). Emitted grouped, ahead of any W eviction."""
                w_in = w_tiles[a]
                absdump = ws.tile([128, D_IN], F32, tag="w_dump",
                                  name=f"absdump_{a}")
                ssum = ws.tile([128, 1], F32, tag="w_sum", bufs=4,
                               name=f"ssum_{a}")
                nc.scalar.activation(
                    absdump[:], w_in[:],
                    mybir.ActivationFunctionType.Abs,
                    accum_out=ssum[:],
                )
                scale = ws.tile([128, 1], F32, tag="w_scale", bufs=4,
                                name=f"scale_{a}")
                nc.vector.tensor_scalar(
                    out=scale[:], in0=ssum[:], scalar1=1.0 / D_IN,
                    scalar2=1e-5, op0=mybir.AluOpType.mult,
                    op1=mybir.AluOpType.max,
                )
                hpos = ws.tile([128, 1], F32, tag="w_hpos", bufs=4,
                               name=f"hp_{a}")
                hneg = ws.tile([128, 1], F32, tag="w_hneg", bufs=4,
                               name=f"hn_{a}")
                nc.vector.tensor_scalar_mul(hpos[:], scale[:], 0.5)
                nc.vector.tensor_scalar_mul(hneg[:], scale[:], -0.5)
                w_scales[a] = (scale, hpos, hneg)

            def w_build(a):
                """Ternary build + PE transpose + ACT eviction for W
                chunk a: (w > 0.5*scale)*scale - (w < -0.5*scale)*scale,
                bit-identical to the reference at the 0.5 boundary."""
                w_in = w_tiles.pop(a)
                scale, hpos, hneg = w_scales.pop(a)
                pos = ws.tile([128, D_IN], F32, tag="w_pos", bufs=2,
                              name=f"pos_{a}")
                nc.vector.tensor_scalar(
                    out=pos[:], in0=w_in[:], scalar1=hpos[:], scalar2=scale[:],
                    op0=mybir.AluOpType.is_gt, op1=mybir.AluOpType.mult,
                )
                neg = ws.tile([128, D_IN], F32, tag="w_neg", bufs=2,
                              name=f"neg_{a}")
                nc.vector.tensor_scalar(
                    out=neg[:], in0=w_in[:], scalar1=hneg[:], scalar2=scale[:],
                    op0=mybir.AluOpType.is_lt, op1=mybir.AluOpType.mult,
                )
                # weff in bf16 from here on: the ternary decision is made
                # in exact fp32 above; bf16 only rounds the +/-scale
                # magnitude (same single rounding the old f32r->bf16
                # eviction applied), and it halves the bytes through the
                # transpose+eviction chain that gates wts[].
                weff = ws.tile([128, D_IN], BF16, tag="w_eff", bufs=2,
                               name=f"weff_{a}")
                nc.vector.tensor_sub(weff[:], pos[:], neg[:])

                n_idx, o_off = divmod(a * 128, N_SLICE)
                for kg in range(K_SUB // TGRP):
                    pt = ps.tile([128, TGRP * 128], BF16, tag="wtps", bufs=2,
                                 name=f"wpt_{a}_{kg}")
                    for j in range(TGRP):
                        k = kg * TGRP + j
                        nc.tensor.transpose(
                            pt[:, j * 128 : (j + 1) * 128],
                            weff[:, k * 128 : (k + 1) * 128],
                            ident[:],
                        )
                    half = TGRP // 2
                    dst = wts[n_idx][:, kg * TGRP : (kg + 1) * TGRP,
                                     o_off : o_off + 128]
                    src = pt[:].rearrange("p (g c) -> p g c", g=TGRP)
                    nc.scalar.copy(dst[:, :half], src[:, :half])
                    nc.scalar.copy(dst[:, half:], src[:, half:])

            def x_load(c):
                """Start 4 per-msub SWDGE DMAs for x chunk c (512KiB
                each) on the otherwise-idle gpsimd/Q7 path. (A single
                2MiB DMA per chunk measured 10us slower end-to-end.)"""
                tiles = []
                for m in range(MSUB):
                    xm = xs.tile([128, K_SUB, 128], BF16, tag=f"x{m}",
                                 bufs=4, name=f"x{m}_{c}")
                    nc.gpsimd.dma_start(xm[:], x_d[c, :, m])
                    tiles.append(xm)
                return tiles

            def mm_group(c, m, n, xk):
                """One accumulation group + eviction + 256KB y store.
                Evictions split by n-slice across ACT and DVE."""
                acc = ymm.tile([128, N_SLICE], F32, tag="y_ps",
                               name=f"acc_{c}_{m}_{n}", bufs=5)
                lhs = xk[m]
                for k in range(K_SUB):
                    nc.tensor.matmul(
                        acc[:],
                        lhs[:, k, :],
                        wts[n][:, k, :],
                        start=(k == 0),
                        stop=(k == K_SUB - 1),
                    )
                y_sb = ysp.tile([128, N_SLICE], F32, tag="y_sb",
                                name=f"y_sb_{c}_{m}_{n}", bufs=4)
                if n == 0:
                    nc.scalar.copy(y_sb[:], acc[:])
                else:
                    nc.vector.tensor_copy(y_sb[:], acc[:])
                nc.sync.dma_start(
                    y_d[(c * MSUB + m) * 128 : (c * MSUB + m + 1) * 128,
                        n * N_SLICE : (n + 1) * N_SLICE],
                    y_sb[:],
                )

            # Emission schedule. Lead-in: w DMAs on two rings, x chunk 0
            # prefetching on the Q7 path, warm-up matmuls bridging the PE
            # until wts[0] (W chunks 0-3) is quantized. Chunks 0-1 run
            # n=0 before any n=1 group so wts[1] (W chunks 4-7, DVE-
            # serial quant) has time to land.
            for a in range(4):
                w_dma(a)
            xk0 = x_load(0)
            warmup(12)
            for a in range(4):
                w_abs(a)
                w_build(a)
            xk1 = x_load(1)
            # bridge the PE through the DVE-serial quant chain (~26us)
            # so the HAM clock gate never re-throttles before the stream
            warmup(64)
            for m in range(MSUB):
                mm_group(0, m, 0, xk0)
                w_dma(4 + m)
            xk2 = x_load(2)
            for m in range(MSUB):
                mm_group(1, m, 0, xk1)
                # small warm burst ahead of each data-gated transpose
                # cluster: shortens the PE idle gap below the 3.4us HAM
                # MID window so the clock stays at 8/8 (fill behind a
                # stalled instruction is useless — FIFO — so it must sit
                # in front of the stall point; 14 MMs ~= the measured
                # 3.0-3.3us residual wait)
                warmup(14)
                w_abs(4 + m)
                w_build(4 + m)
            for m in range(MSUB):
                mm_group(0, m, 1, xk0)
            xk3 = x_load(3)
            for m in range(MSUB):
                mm_group(1, m, 1, xk1)
            xk_tiles = {2: xk2, 3: xk3}
            for c in range(2, NCHUNK):
                if c + 2 < NCHUNK:
                    xk_tiles[c + 2] = x_load(c + 2)
                for m in range(MSUB):
                    mm_group(c, m, 0, xk_tiles[c])
                    mm_group(c, m, 1, xk_tiles[c])

    nc.compile()
    return nc


_NC_CACHE = None


def _get_nc():
    global _NC_CACHE
    if _NC_CACHE is None:
        _NC_CACHE = _build()
    return _NC_CACHE


def kernel(x: np.ndarray, weight: np.ndarray, _trace: bool = False):
    assert x.shape == (B, S, D_IN) and weight.shape == (D_OUT, D_IN)
    # Host layout prep: bf16 [chunk, k_in, msub, k_sub, row] so each
    # (chunk, msub) DMAs in as ready-to-use stationary tiles (k on
    # partitions), contiguous per partition.
    x_flat = np.asarray(x, dtype=np.float32).reshape(R, D_IN)
    xr = np.ascontiguousarray(
        x_flat.reshape(NCHUNK, MSUB, 128, K_SUB, 128)
        .transpose(0, 4, 1, 3, 2)
        .astype(ml_dtypes.bfloat16)
    )
    in_maps = [
        {
            "x": xr,
            "w": np.ascontiguousarray(
                weight[c * O : (c + 1) * O], dtype=np.float32
            ),
        }
        for c in range(NCORES)
    ]
    nc = _get_nc()
    res = run_bass_kernel_spmd(
        nc, in_maps, core_ids=list(range(NCORES)), trace=_trace
    )
    y = np.concatenate([res.results[c]["y"] for c in range(NCORES)], axis=1)
    out = y.reshape(B, S, D_OUT)
    if _trace:
        return out, res
    return out
